# revision 20
# baseline (speedup 1.0000x reference)
"""MoE layer (B=8192, D=1024, E=8, top-2, H=2048) on 8 TRN2 NeuronCores.

Strategy (expert-parallel, mixed bf16 / fp8-DoubleRow):
  - Host: gate logits = x @ Wg (fp32), exact top-2 (jax tie-break semantics),
    softmax over the 2 picks. Core e owns expert e. Each expert's
    (token, expert) pairs are split by gate weight: the n8[e] lowest-gate
    pairs run in fp8 (e4m3, perf_mode=DoubleRow: 2 contraction tiles per
    matmul at the same ~215.8ns N=512 duration as bf16 => half the matmul
    count, 2.0x per unit work), the rest in bf16. n8[e] = counts[e] -
    C16 makes every core's bf16 workload exactly C16 columns, so per-core
    cost is uniform despite expert imbalance. C16 is the smallest value
    whose total gate^2 mass in fp8 stays under an error budget
    (rel err ~1.79e-2 < 2e-2 gate; low-gate pairs' fp8 error is cheap).
  - Device (SPMD, identical program all cores): y = relu(x @ W1 + b1) @ W2
    with fp32 PSUM accumulation. bf16 path: plain matmuls (215.8ns/MM).
    fp8 path: x, 16*W1, 64*W2 in e4m3; h written as e4m3 with scale 16
    (psum is already 16*(x@W1), so relu is scale-free: max(psum+16*b1, 0)),
    alternating between the ACT engine (~620ns fixed per call) and the DVE
    (tensor_scalar add+max) so the relu stream never paces the short fp8
    matmul groups. Weights resident in SBUF; column chunks <=512; h
    double-buffered so chunk boundaries don't stall the PE.
  - Host: weighted combine out[tok] = sum_k gate * (y + b2[e]); fp8
    columns are descaled by 1/(16*64).
"""

import os

import numpy as np
import ml_dtypes

B, D, E, TOP_K = 8192, 1024, 8, 2
H = 2 * D
P = 128
CHUNK = 512

KD = D // P  # 8 contraction tiles for mm1 (over D)
MH = H // P  # 16 output tiles for mm1 / contraction tiles for mm2 (over H)
MD = D // P  # 8 output tiles for mm2 (over D)

S_W1 = 16.0  # fp8 scales (powers of 2). S_W1 == S_H makes the fp8 relu
S_H = 16.0   # scale-free: psum = 16*(x@W1), h8 = max(psum + 16*b1, 0).
S_W2 = 64.0
G2_BUDGET = 0.110  # max fraction of total gate^2 mass routed through fp8
# (sim-calibrated: rel err ~1.79e-2 on the reference data, HW matches sim
# to 4 digits; the 2e-2 gate keeps ~11% margin)

_BF16 = np.dtype(ml_dtypes.bfloat16)
_F8 = np.dtype(ml_dtypes.float8_e4m3)  # IEEE-style e4m3: max 240, like TRN

LAST_RESULTS = None  # BassKernelResults of the most recent run (for test harness)


def _chunk_sizes(n):
    """Split n columns into matmul chunks <=512, avoiding tiny tails."""
    chunks = [CHUNK] * (n // CHUNK)
    tail = n % CHUNK
    if tail >= 128 or not chunks:
        if tail:
            chunks.append(tail)
    elif tail:
        last = chunks.pop() + tail
        chunks += [last - last // 2, last // 2]
    return chunks


def _build_program(C16, C8):
    import concourse.bacc as bacc
    import concourse.mybir as mybir
    import concourse.tile as tile
    from concourse.bass import ts

    DR = mybir.MatmulPerfMode.DoubleRow

    nc = bacc.Bacc("TRN2", target_bir_lowering=False, debug=False)
    bf16 = mybir.dt.bfloat16
    f16 = mybir.dt.float16
    f8 = mybir.dt.float8e4
    f32 = mybir.dt.float32

    xt16_d = nc.dram_tensor("xt16", (D, C16), bf16, kind="ExternalInput").ap()
    xt8_d = nc.dram_tensor("xt8", (D, C8), f8, kind="ExternalInput").ap()
    w1h_d = nc.dram_tensor("w1h", (D, H), bf16, kind="ExternalInput").ap()
    w2h_d = nc.dram_tensor("w2h", (H, D), bf16, kind="ExternalInput").ap()
    w18_d = nc.dram_tensor("w18", (D, H), f8, kind="ExternalInput").ap()
    w28_d = nc.dram_tensor("w28", (H, D), f8, kind="ExternalInput").ap()
    b1h_d = nc.dram_tensor("b1h", (P, MH), f32, kind="ExternalInput").ap()
    b18_d = nc.dram_tensor("b18", (P, MH), f32, kind="ExternalInput").ap()
    yt16_d = nc.dram_tensor("yt16", (D, C16), f16, kind="ExternalOutput").ap()
    yt8_d = nc.dram_tensor("yt8", (D, C8), f16, kind="ExternalOutput").ap()
    yt16_r = yt16_d.rearrange("(m p) c -> p m c", p=P)
    yt8_r = yt8_d.rearrange("(m p) c -> p m c", p=P)

    with tile.TileContext(nc) as tc:
        with (
            tc.tile_pool(name="weights", bufs=1) as wpool,
            tc.tile_pool(name="xin", bufs=1) as xpool,
            tc.tile_pool(name="hbuf", bufs=2) as hpool,
            tc.tile_pool(name="ystage", bufs=3) as ypool,
            tc.tile_pool(name="ps", bufs=8, space="PSUM") as pspool,
        ):
            # Weights allocated first: keeps their SBUF offsets 2KB-aligned
            # (the PE fast weight-load path degrades ~+43ns/MM otherwise).
            w1h_sb = wpool.tile([P, KD, H], bf16, name="w1hsb")
            w2h_sb = wpool.tile([P, MH, D], bf16, name="w2hsb")
            w18_sb = wpool.tile([P, KD, H], f8, name="w18sb")
            w28_sb = wpool.tile([P, MH, D], f8, name="w28sb")
            b1h_sb = wpool.tile([P, MH], f32, name="b1hsb")
            b18_sb = wpool.tile([P, MH], f32, name="b18sb")
            xt16_sb = xpool.tile([P, KD, C16], bf16, name="xt16sb")
            xt8_sb = xpool.tile([P, KD, C8], f8, name="xt8sb")
            xt16_r = xt16_d.rearrange("(ko p) c -> p ko c", p=P)
            xt8_r = xt8_d.rearrange("(ko p) c -> p ko c", p=P)
            w1h_r = w1h_d.rearrange("(ko p) h -> p ko h", p=P)
            w2h_r = w2h_d.rearrange("(ko p) d -> p ko d", p=P)
            w18_r = w18_d.rearrange("(ko p) h -> p ko h", p=P)
            w28_r = w28_d.rearrange("(ko p) d -> p ko d", p=P)

            # DMA issue order == need order (each dma_start costs ~600ns of
            # issue time; two HWDGE issue engines: scalar carries the first
            # x chunk, sync everything else, so the ramp is not serialized
            # behind one stream). The scalar engine's first relu isn't
            # needed until ~10us in, after its 9 issues have drained.
            nc.scalar.dma_start(xt16_sb[:, 0, 0:256], xt16_r[:, 0, 0:256])
            for k in range(0, KD, 2):  # w1h m-group 0 on scalar: 4 x 64KB
                nc.scalar.dma_start(
                    w1h_sb[:, k : k + 2, 0:128], w1h_r[:, k : k + 2, 0:128]
                )
            for k in range(1, KD):  # rest of chunk-0 x: 7 x 64KB
                nc.scalar.dma_start(xt16_sb[:, k, 0:256], xt16_r[:, k, 0:256])

            def dma(dst, src):
                nc.sync.dma_start(dst, src)

            # b1h first: 8KB, chunk-0 ACTs need it ~10us in; a late bias
            # stalls ACT -> held PSUM slots -> PE slot exhaustion.
            dma(b1h_sb, b1h_d)
            for k in range(0, KD, 2):  # w1h m-group 1
                dma(w1h_sb[:, k : k + 2, 128:256], w1h_r[:, k : k + 2, 128:256])
            for k in range(0, KD, 2):  # w1h m-groups 2..3
                dma(w1h_sb[:, k : k + 2, 256:512], w1h_r[:, k : k + 2, 256:512])
            for k in range(KD):  # w1h m-groups 4..9: 8 x 192KB
                dma(w1h_sb[:, k, 512:1280], w1h_r[:, k, 512:1280])
            for k in range(KD):  # w1h m-groups 10..15
                dma(w1h_sb[:, k, 1280:H], w1h_r[:, k, 1280:H])
            for k in range(0, MH, 2):  # w2h: 8 x 512KB (needed from ~28us)
                dma(w2h_sb[:, k : k + 2], w2h_r[:, k : k + 2])
            if C16 > 256:  # x bf16 remaining cols (needed ~40us in)
                for k in range(0, KD, 2):
                    dma(xt16_sb[:, k : k + 2, 256:C16],
                        xt16_r[:, k : k + 2, 256:C16])
            for k in range(0, KD, 2):  # fp8 weights: needed after bf16 chunks
                dma(w18_sb[:, k : k + 2], w18_r[:, k : k + 2])
            for k in range(0, MH, 4):
                dma(w28_sb[:, k : k + 4], w28_r[:, k : k + 4])
            dma(b18_sb, b18_d)
            for k in range(0, KD, 4):  # fp8 x columns
                dma(xt8_sb[:, k : k + 4], xt8_r[:, k : k + 4])

            # chunk list: (is8, off, tw); bf16 chunks first (their weights
            # arrive first), then fp8; smallest fp8 chunk last so the final
            # copy+DMA trail is short.
            chunks = []
            off = 0
            c16_sizes = _chunk_sizes(C16)
            if C16 > 768:
                # Narrow first chunk: halves the x bytes the ramp waits on.
                c16_sizes = [256] + _chunk_sizes(C16 - 256)
            for tw in c16_sizes:
                chunks.append((False, off, tw))
                off += tw
            f8_chunks = []
            off = 0
            for tw in _chunk_sizes(C8):
                f8_chunks.append((True, off, tw))
                off += tw
            f8_chunks.sort(key=lambda c: -c[2])
            chunks += f8_chunks

            def mm2_phase(is8, off, tw, h_sb, last):
                # k2-outer: all output banks in a half accumulate together so
                # each w2[k2] slice is consumed as it lands, and only 4 PSUM
                # banks are held at a time. Each half's 4 output tiles are
                # staged into one SBUF tile and written with a single DMA
                # (sync-queue issue count is the scarce resource). On the
                # last chunk the second half runs m2-outer so its copies
                # overlap its own matmul stream and only the final copy+DMA
                # trails.
                w2_sb, yt_r = (w28_sb, yt8_r) if is8 else (w2h_sb, yt16_r)

                def mm2(py, m2, k2, start, stop):
                    if is8:
                        nc.tensor.matmul(
                            py[:, :tw],
                            w2_sb[:, 2 * k2 : 2 * k2 + 2, ts(m2, P)],
                            h_sb[:, 2 * k2 : 2 * k2 + 2, :tw],
                            start=start,
                            stop=stop,
                            perf_mode=DR,
                        )
                    else:
                        nc.tensor.matmul(
                            py[:, :tw],
                            w2_sb[:, k2, ts(m2, P)],
                            h_sb[:, k2, :tw],
                            start=start,
                            stop=stop,
                        )

                nk2 = MH // 2 if is8 else MH
                nh = MD // 2
                for hi in range(2):
                    m2s = range(hi * nh, (hi + 1) * nh)
                    if last and hi == 1:
                        # m2-outer: copies overlap the matmul stream; one
                        # staged DMA writes the half.
                        y_sb = ypool.tile([P, nh, CHUNK], f16, tag="y")
                        for j, m2 in enumerate(m2s):
                            py = pspool.tile([P, CHUNK], f32, tag="ps",
                                             name="py")
                            for k2 in range(nk2):
                                mm2(py, m2, k2, k2 == 0, k2 == nk2 - 1)
                            nc.vector.tensor_copy(y_sb[:, j, :tw], py[:, :tw])
                        nc.sync.dma_start(
                            yt_r[:, hi * nh : (hi + 1) * nh, off : off + tw],
                            y_sb[:, :, :tw],
                        )
                    else:
                        y_sb = ypool.tile([P, nh, CHUNK], f16, tag="y")
                        pys = {
                            m2: pspool.tile([P, CHUNK], f32, tag="ps",
                                            name=f"py{m2}")
                            for m2 in m2s
                        }
                        for k2 in range(nk2):
                            for m2 in m2s:
                                mm2(pys[m2], m2, k2, k2 == 0, k2 == nk2 - 1)
                        for j, m2 in enumerate(m2s):
                            nc.vector.tensor_copy(y_sb[:, j, :tw],
                                                  pys[m2][:, :tw])
                        nc.sync.dma_start(
                            yt_r[:, hi * nh : (hi + 1) * nh, off : off + tw],
                            y_sb[:, :, :tw],
                        )

            # PE warmup: junk matmuls on a memset tile run while the first
            # weight/activation DMAs land, so the HAM clock gate is already
            # at 8/8 when real matmuls start.
            warm_sb = xpool.tile([P, P], bf16, name="warm")
            nc.vector.memset(warm_sb, 0.0)
            warm_ps = pspool.tile([P, P], f32, tag="ps", name="warm_ps")
            for _ in range(44):
                nc.tensor.matmul(warm_ps, warm_sb, warm_sb, start=True, stop=True)

            add_op = mybir.AluOpType.add
            max_op = mybir.AluOpType.max

            for ci, (is8, off, tw) in enumerate(chunks):
                last = ci == len(chunks) - 1
                if is8:
                    h_sb = hpool.tile([P, MH, CHUNK], f8, tag="h", name="h8")
                    for m in range(MH):
                        ph = pspool.tile([P, CHUNK], f32, tag="ps", name="ph")
                        for kp in range(KD // 2):
                            nc.tensor.matmul(
                                ph[:, :tw],
                                w18_sb[:, 2 * kp : 2 * kp + 2, ts(m, P)],
                                xt8_sb[:, 2 * kp : 2 * kp + 2, off : off + tw],
                                start=(kp == 0),
                                stop=(kp == KD // 2 - 1),
                                perf_mode=DR,
                            )
                        # psum = 16*(x@W1); h8 = max(psum + 16*b1, 0).
                        # Alternate ACT / DVE so neither engine's ~600ns
                        # fixed cost paces the 4-matmul (~640ns) groups.
                        if m % 2 == 0:
                            nc.scalar.activation(
                                h_sb[:, m, :tw],
                                ph[:, :tw],
                                mybir.ActivationFunctionType.Relu,
                                bias=b18_sb[:, m : m + 1],
                            )
                        else:
                            nc.vector.tensor_scalar(
                                h_sb[:, m, :tw],
                                ph[:, :tw],
                                b18_sb[:, m : m + 1],
                                0.0,
                                add_op,
                                max_op,
                            )
                else:
                    h_sb = hpool.tile([P, MH, CHUNK], bf16, tag="h", name="h16")
                    for m in range(MH):
                        ph = pspool.tile([P, CHUNK], f32, tag="ps", name="ph")
                        for k in range(KD):
                            nc.tensor.matmul(
                                ph[:, :tw],
                                w1h_sb[:, k, ts(m, P)],
                                xt16_sb[:, k, off : off + tw],
                                start=(k == 0),
                                stop=(k == KD - 1),
                            )
                        nc.scalar.activation(
                            h_sb[:, m, :tw],
                            ph[:, :tw],
                            mybir.ActivationFunctionType.Relu,
                            bias=b1h_sb[:, m : m + 1],
                        )
                        if ci == 0 and m < 10:
                            # Fill DMA-ramp bubbles with dependency-free
                            # matmuls so the HAM clock gate stays at 8/8
                            # while chunk-0 weights stream in. Densest
                            # early, where arrival lags consumption most.
                            for _ in range(8 if m < 4 else 4):
                                nc.tensor.matmul(
                                    warm_ps, warm_sb, warm_sb,
                                    start=True, stop=True,
                                )
                mm2_phase(is8, off, tw, h_sb, last)
    nc.finalize()
    return nc


def _route(x, Wg):
    """Exact reference gating on host: top-2 of clean fp32 logits (jax
    tie-break: lower index first), softmax over the two picks."""
    logits = x @ Wg  # [B, E] fp32
    order = np.argsort(-logits, axis=1, kind="stable")[:, :TOP_K]  # [B, 2]
    top_vals = np.take_along_axis(logits, order, axis=1)
    ex = np.exp(top_vals - top_vals[:, :1])  # top_vals sorted desc -> max first
    gates = (ex / ex.sum(axis=1, keepdims=True)).astype(np.float32)  # [B, 2]
    return order, gates


def _pick_c16(counts, g2_prefix, total_g2):
    """Smallest C16 (multiple of 16) whose fp8 allocation n8[e] =
    max(0, counts[e] - C16) keeps the fp8 gate^2 mass under budget."""

    def frac(c16):
        used = 0.0
        for e in range(E):
            n8 = max(0, int(counts[e]) - c16)
            if n8:
                used += g2_prefix[e][min(n8, len(g2_prefix[e])) - 1]
        return used / total_g2

    # Prefer an exact multiple of 512 (bf16 chunks with no sub-512 tail).
    c512 = int(-(-counts.max() // 512) * 512) - 512
    while c512 > 0 and frac(c512) <= G2_BUDGET:
        c512 -= 512
    c512 += 512
    best = c512 if frac(c512) <= G2_BUDGET else None
    for c16 in range(1024, int(counts.max()) + 16, 16):
        if best is not None and c16 >= best:
            break
        if frac(c16) <= G2_BUDGET:
            # accept a non-512-multiple only if it saves >48 bf16 columns
            if best is None or best - c16 > 48:
                best = c16
            break
    if best is None:
        best = int(-(-int(counts.max()) // 16) * 16)
    return best


def _to_f8(a, scale):
    return np.clip(np.asarray(a, np.float32) * scale, -240.0, 240.0).astype(_F8)


def kernel(x, Wg, W1, b1, W2, b2):
    x = np.ascontiguousarray(np.asarray(x, dtype=np.float32))
    Wg = np.asarray(Wg, dtype=np.float32)
    W1 = np.asarray(W1, dtype=np.float32)
    b1 = np.asarray(b1, dtype=np.float32)
    W2 = np.asarray(W2, dtype=np.float32)
    b2 = np.asarray(b2, dtype=np.float32)

    order, gates = _route(x, Wg)

    expert_flat = order.reshape(-1)  # [2B]; pair p belongs to token p//2
    gate_flat = gates.reshape(-1)
    counts = np.bincount(expert_flat, minlength=E)

    # Per-expert pairs sorted by gate ascending (stable).
    pair_idx_of = []
    g2_prefix = []
    for e in range(E):
        pidx = np.where(expert_flat == e)[0]
        si = np.argsort(gate_flat[pidx], kind="stable")
        pidx = pidx[si]
        pair_idx_of.append(pidx)
        g2_prefix.append(np.cumsum(gate_flat[pidx].astype(np.float64) ** 2))
    total_g2 = float((gate_flat.astype(np.float64) ** 2).sum())

    C16 = _pick_c16(counts, g2_prefix, total_g2)
    n8 = np.maximum(0, counts - C16)
    C8 = int(-(-max(int(n8.max()), 16) // 16) * 16)  # pad to mult of 16

    # Per-pair placement for the combine step, and per-core inputs.
    core_of_pair = expert_flat  # core e == expert e
    col_of_pair = np.empty(2 * B, dtype=np.int64)
    is8_of_pair = np.zeros(2 * B, dtype=bool)
    xT = np.ascontiguousarray(x.T)  # [D, B]
    in_maps = []
    for e in range(E):
        pidx = pair_idx_of[e]
        p8, p16 = pidx[: n8[e]], pidx[n8[e] :]
        col_of_pair[p8] = np.arange(len(p8))
        is8_of_pair[p8] = True
        col_of_pair[p16] = np.arange(len(p16))

        xg16 = np.zeros((D, C16), dtype=_BF16)
        xg16[:, : len(p16)] = xT[:, p16 // 2].astype(_BF16)
        xg8 = np.zeros((D, C8), dtype=_F8)
        xg8[:, : len(p8)] = _to_f8(xT[:, p8 // 2], 1.0)

        in_maps.append({
            "xt16": xg16,
            "xt8": xg8,
            "w1h": W1[e].astype(_BF16),
            "w2h": W2[e].astype(_BF16),
            "w18": _to_f8(W1[e], S_W1),
            "w28": _to_f8(W2[e], S_W2),
            "b1h": np.ascontiguousarray(b1[e].reshape(MH, P).T),
            "b18": np.ascontiguousarray((S_H * b1[e]).reshape(MH, P).T),
        })

    nc = _build_program(C16, C8)

    from concourse.bass_utils import run_bass_kernel_spmd

    trace = os.environ.get("MOE_TRACE") == "1"
    kwargs = {}
    if trace:
        kwargs = dict(trace=True, trace_cores=list(range(E)))
    try:
        res = run_bass_kernel_spmd(nc, in_maps, core_ids=list(range(E)), **kwargs)
    except Exception:  # wedged accelerator: reset once and retry untraced
        try:
            import ctypes

            lib = ctypes.CDLL("/opt/axon/libaxon_pjrt.so")
            lib.axon_reset.restype = ctypes.c_int64
            lib.axon_reset()
        except OSError:
            pass
        res = run_bass_kernel_spmd(nc, in_maps, core_ids=list(range(E)))
    global LAST_RESULTS
    LAST_RESULTS = res

    Y16 = np.stack([r["yt16"] for r in res.results]).astype(np.float32)
    Y8 = np.stack([r["yt8"] for r in res.results]).astype(np.float32)
    Y8 *= 1.0 / (S_H * S_W2)

    # Combine: pair p contributes gate_p * (y[:, col_p] + b2[e_p]) to token
    # p//2. Pairs of token b sit at flat positions 2b, 2b+1.
    cols = np.empty((2 * B, D), dtype=np.float32)
    m8 = is8_of_pair
    cols[~m8] = Y16[core_of_pair[~m8], :, col_of_pair[~m8]]
    cols[m8] = Y8[core_of_pair[m8], :, col_of_pair[m8]]
    weighted = (cols + b2[expert_flat]) * gate_flat[:, None]
    out = weighted[0::2] + weighted[1::2]
    return np.ascontiguousarray(out, dtype=np.float32)


# revision 21
# speedup vs baseline: 1.0352x; 1.0352x over previous
"""MoE layer (B=8192, D=1024, E=8, top-2, H=2048) on 8 TRN2 NeuronCores.

Strategy (expert-parallel, mixed bf16 / fp8-DoubleRow):
  - Host: gate logits = x @ Wg (fp32), exact top-2 (jax tie-break semantics),
    softmax over the 2 picks. Core e owns expert e. Each expert's
    (token, expert) pairs are split by gate weight: the n8[e] lowest-gate
    pairs run in fp8 (e4m3, perf_mode=DoubleRow: 2 contraction tiles per
    matmul at the same ~215.8ns N=512 duration as bf16 => half the matmul
    count, 2.0x per unit work), the rest in bf16. n8[e] = counts[e] -
    C16 makes every core's bf16 workload exactly C16 columns, so per-core
    cost is uniform despite expert imbalance. C16 is the smallest value
    whose total gate^2 mass in fp8 stays under an error budget
    (rel err ~1.79e-2 < 2e-2 gate; low-gate pairs' fp8 error is cheap).
  - Device (SPMD, identical program all cores): y = relu(x @ W1 + b1) @ W2
    with fp32 PSUM accumulation. bf16 path: plain matmuls (215.8ns/MM).
    fp8 path: x, 16*W1, 64*W2 in e4m3; h written as e4m3 with scale 16
    (psum is already 16*(x@W1), so relu is scale-free: max(psum+16*b1, 0)),
    alternating between the ACT engine (~620ns fixed per call) and the DVE
    (tensor_scalar add+max) so the relu stream never paces the short fp8
    matmul groups. Weights resident in SBUF; column chunks <=512; h
    double-buffered so chunk boundaries don't stall the PE.
  - Host: weighted combine out[tok] = sum_k gate * (y + b2[e]); fp8
    columns are descaled by 1/(16*64).
"""

import os

import numpy as np
import ml_dtypes

B, D, E, TOP_K = 8192, 1024, 8, 2
H = 2 * D
P = 128
CHUNK = 512

KD = D // P  # 8 contraction tiles for mm1 (over D)
MH = H // P  # 16 output tiles for mm1 / contraction tiles for mm2 (over H)
MD = D // P  # 8 output tiles for mm2 (over D)

S_W1 = 16.0  # fp8 scales (powers of 2). S_W1 == S_H makes the fp8 relu
S_H = 16.0   # scale-free: psum = 16*(x@W1), h8 = max(psum + 16*b1, 0).
S_W2 = 64.0
G2_BUDGET = 0.110  # max fraction of total gate^2 mass routed through fp8
# (sim-calibrated: rel err ~1.79e-2 on the reference data, HW matches sim
# to 4 digits; the 2e-2 gate keeps ~11% margin)

_BF16 = np.dtype(ml_dtypes.bfloat16)
_F8 = np.dtype(ml_dtypes.float8_e4m3)  # IEEE-style e4m3: max 240, like TRN

LAST_RESULTS = None  # BassKernelResults of the most recent run (for test harness)


def _chunk_sizes(n):
    """Split n columns into matmul chunks <=512, avoiding tiny tails."""
    chunks = [CHUNK] * (n // CHUNK)
    tail = n % CHUNK
    if tail >= 128 or not chunks:
        if tail:
            chunks.append(tail)
    elif tail:
        last = chunks.pop() + tail
        chunks += [last - last // 2, last // 2]
    return chunks


def _build_program(C16, C8):
    import concourse.bacc as bacc
    import concourse.mybir as mybir
    import concourse.tile as tile
    from concourse.bass import ts

    DR = mybir.MatmulPerfMode.DoubleRow

    nc = bacc.Bacc("TRN2", target_bir_lowering=False, debug=False)
    bf16 = mybir.dt.bfloat16
    f16 = mybir.dt.float16
    f8 = mybir.dt.float8e4
    f32 = mybir.dt.float32

    xt16_d = nc.dram_tensor("xt16", (D, C16), bf16, kind="ExternalInput").ap()
    xt8_d = nc.dram_tensor("xt8", (D, C8), f8, kind="ExternalInput").ap()
    w1h_d = nc.dram_tensor("w1h", (D, H), bf16, kind="ExternalInput").ap()
    w2h_d = nc.dram_tensor("w2h", (H, D), bf16, kind="ExternalInput").ap()
    w18_d = nc.dram_tensor("w18", (D, H), f8, kind="ExternalInput").ap()
    w28_d = nc.dram_tensor("w28", (H, D), f8, kind="ExternalInput").ap()
    b1h_d = nc.dram_tensor("b1h", (P, MH), f32, kind="ExternalInput").ap()
    b18_d = nc.dram_tensor("b18", (P, MH), f32, kind="ExternalInput").ap()
    yt16_d = nc.dram_tensor("yt16", (D, C16), f16, kind="ExternalOutput").ap()
    yt8_d = nc.dram_tensor("yt8", (D, C8), f16, kind="ExternalOutput").ap()
    yt16_r = yt16_d.rearrange("(m p) c -> p m c", p=P)
    yt8_r = yt8_d.rearrange("(m p) c -> p m c", p=P)

    with tile.TileContext(nc) as tc:
        with (
            tc.tile_pool(name="weights", bufs=1) as wpool,
            tc.tile_pool(name="xin", bufs=1) as xpool,
            tc.tile_pool(name="hbuf", bufs=2) as hpool,
            tc.tile_pool(name="ystage", bufs=3) as ypool,
            tc.tile_pool(name="ps", bufs=8, space="PSUM") as pspool,
        ):
            # Weights allocated first: keeps their SBUF offsets 2KB-aligned
            # (the PE fast weight-load path degrades ~+43ns/MM otherwise).
            w1h_sb = wpool.tile([P, KD, H], bf16, name="w1hsb")
            w2h_sb = wpool.tile([P, MH, D], bf16, name="w2hsb")
            w18_sb = wpool.tile([P, KD, H], f8, name="w18sb")
            w28_sb = wpool.tile([P, MH, D], f8, name="w28sb")
            b1h_sb = wpool.tile([P, MH], f32, name="b1hsb")
            b18_sb = wpool.tile([P, MH], f32, name="b18sb")
            xt16_sb = xpool.tile([P, KD, C16], bf16, name="xt16sb")
            xt8_sb = xpool.tile([P, KD, C8], f8, name="xt8sb")
            xt16_r = xt16_d.rearrange("(ko p) c -> p ko c", p=P)
            xt8_r = xt8_d.rearrange("(ko p) c -> p ko c", p=P)
            w1h_r = w1h_d.rearrange("(ko p) h -> p ko h", p=P)
            w2h_r = w2h_d.rearrange("(ko p) d -> p ko d", p=P)
            w18_r = w18_d.rearrange("(ko p) h -> p ko h", p=P)
            w28_r = w28_d.rearrange("(ko p) d -> p ko d", p=P)

            # DMA issue order == need order (each dma_start costs ~600ns of
            # issue time; two HWDGE issue engines: scalar carries the first
            # x chunk, sync everything else, so the ramp is not serialized
            # behind one stream). The scalar engine's first relu isn't
            # needed until ~10us in, after its 9 issues have drained.
            nc.scalar.dma_start(xt16_sb[:, 0, 0:256], xt16_r[:, 0, 0:256])
            nc.scalar.dma_start(xt16_sb[:, 0, 256:CHUNK], xt16_r[:, 0, 256:CHUNK])
            for k in range(0, KD, 2):  # w1h m-group 0 on scalar: 4 x 64KB
                nc.scalar.dma_start(
                    w1h_sb[:, k : k + 2, 0:128], w1h_r[:, k : k + 2, 0:128]
                )
            for k in range(1, KD):
                nc.scalar.dma_start(xt16_sb[:, k, 0:CHUNK], xt16_r[:, k, 0:CHUNK])

            def dma(dst, src):
                nc.sync.dma_start(dst, src)

            # b1h first: 8KB, chunk-0 ACTs need it ~10us in; a late bias
            # stalls ACT -> held PSUM slots -> PE slot exhaustion.
            dma(b1h_sb, b1h_d)
            for k in range(0, KD, 2):  # w1h m-group 1
                dma(w1h_sb[:, k : k + 2, 128:256], w1h_r[:, k : k + 2, 128:256])
            for k in range(0, KD, 2):  # w1h m-groups 2..3
                dma(w1h_sb[:, k : k + 2, 256:512], w1h_r[:, k : k + 2, 256:512])
            for k in range(KD):  # w1h m-groups 4..9: 8 x 192KB
                dma(w1h_sb[:, k, 512:1280], w1h_r[:, k, 512:1280])
            for k in range(KD):  # w1h m-groups 10..15
                dma(w1h_sb[:, k, 1280:H], w1h_r[:, k, 1280:H])
            for k in range(0, MH, 2):  # w2h: 8 x 512KB (needed from ~28us)
                dma(w2h_sb[:, k : k + 2], w2h_r[:, k : k + 2])
            if C16 > CHUNK:  # x bf16 remaining cols (needed ~55us in)
                for k in range(0, KD, 2):
                    dma(xt16_sb[:, k : k + 2, CHUNK:C16],
                        xt16_r[:, k : k + 2, CHUNK:C16])
            for k in range(0, KD, 2):  # fp8 weights: needed after bf16 chunks
                dma(w18_sb[:, k : k + 2], w18_r[:, k : k + 2])
            for k in range(0, MH, 4):
                dma(w28_sb[:, k : k + 4], w28_r[:, k : k + 4])
            dma(b18_sb, b18_d)
            for k in range(0, KD, 4):  # fp8 x columns
                dma(xt8_sb[:, k : k + 4], xt8_r[:, k : k + 4])

            # chunk list: (is8, off, tw); bf16 chunks first (their weights
            # arrive first), then fp8; smallest fp8 chunk last so the final
            # copy+DMA trail is short.
            chunks = []
            off = 0
            for tw in _chunk_sizes(C16):
                chunks.append((False, off, tw))
                off += tw
            f8_chunks = []
            off = 0
            for tw in _chunk_sizes(C8):
                f8_chunks.append((True, off, tw))
                off += tw
            f8_chunks.sort(key=lambda c: -c[2])
            chunks += f8_chunks

            def mm2_phase(is8, off, tw, h_sb, last):
                # k2-outer: all output banks in a half accumulate together so
                # each w2[k2] slice is consumed as it lands, and only 4 PSUM
                # banks are held at a time. Each half's 4 output tiles are
                # staged into one SBUF tile and written with a single DMA
                # (sync-queue issue count is the scarce resource). On the
                # last chunk the second half runs m2-outer so its copies
                # overlap its own matmul stream and only the final copy+DMA
                # trails.
                w2_sb, yt_r = (w28_sb, yt8_r) if is8 else (w2h_sb, yt16_r)

                def mm2(py, m2, k2, start, stop):
                    if is8:
                        nc.tensor.matmul(
                            py[:, :tw],
                            w2_sb[:, 2 * k2 : 2 * k2 + 2, ts(m2, P)],
                            h_sb[:, 2 * k2 : 2 * k2 + 2, :tw],
                            start=start,
                            stop=stop,
                            perf_mode=DR,
                        )
                    else:
                        nc.tensor.matmul(
                            py[:, :tw],
                            w2_sb[:, k2, ts(m2, P)],
                            h_sb[:, k2, :tw],
                            start=start,
                            stop=stop,
                        )

                nk2 = MH // 2 if is8 else MH
                nh = MD // 2
                for hi in range(2):
                    m2s = range(hi * nh, (hi + 1) * nh)
                    if last and hi == 1:
                        # m2-outer: copies overlap the matmul stream; one
                        # staged DMA writes the half.
                        y_sb = ypool.tile([P, nh, CHUNK], f16, tag="y")
                        for j, m2 in enumerate(m2s):
                            py = pspool.tile([P, CHUNK], f32, tag="ps",
                                             name="py")
                            for k2 in range(nk2):
                                mm2(py, m2, k2, k2 == 0, k2 == nk2 - 1)
                            nc.vector.tensor_copy(y_sb[:, j, :tw], py[:, :tw])
                        nc.sync.dma_start(
                            yt_r[:, hi * nh : (hi + 1) * nh, off : off + tw],
                            y_sb[:, :, :tw],
                        )
                    else:
                        y_sb = ypool.tile([P, nh, CHUNK], f16, tag="y")
                        pys = {
                            m2: pspool.tile([P, CHUNK], f32, tag="ps",
                                            name=f"py{m2}")
                            for m2 in m2s
                        }
                        for k2 in range(nk2):
                            for m2 in m2s:
                                mm2(pys[m2], m2, k2, k2 == 0, k2 == nk2 - 1)
                        for j, m2 in enumerate(m2s):
                            nc.vector.tensor_copy(y_sb[:, j, :tw],
                                                  pys[m2][:, :tw])
                        nc.sync.dma_start(
                            yt_r[:, hi * nh : (hi + 1) * nh, off : off + tw],
                            y_sb[:, :, :tw],
                        )

            # PE warmup: junk matmuls on a memset tile run while the first
            # weight/activation DMAs land, so the HAM clock gate is already
            # at 8/8 when real matmuls start.
            warm_sb = xpool.tile([P, P], bf16, name="warm")
            nc.vector.memset(warm_sb, 0.0)
            warm_ps = pspool.tile([P, P], f32, tag="ps", name="warm_ps")
            for _ in range(44):
                nc.tensor.matmul(warm_ps, warm_sb, warm_sb, start=True, stop=True)

            add_op = mybir.AluOpType.add
            max_op = mybir.AluOpType.max

            for ci, (is8, off, tw) in enumerate(chunks):
                last = ci == len(chunks) - 1
                if is8:
                    h_sb = hpool.tile([P, MH, CHUNK], f8, tag="h", name="h8")
                    for m in range(MH):
                        ph = pspool.tile([P, CHUNK], f32, tag="ps", name="ph")
                        for kp in range(KD // 2):
                            nc.tensor.matmul(
                                ph[:, :tw],
                                w18_sb[:, 2 * kp : 2 * kp + 2, ts(m, P)],
                                xt8_sb[:, 2 * kp : 2 * kp + 2, off : off + tw],
                                start=(kp == 0),
                                stop=(kp == KD // 2 - 1),
                                perf_mode=DR,
                            )
                        # psum = 16*(x@W1); h8 = max(psum + 16*b1, 0).
                        # Alternate ACT / DVE so neither engine's ~600ns
                        # fixed cost paces the 4-matmul (~640ns) groups.
                        if m % 2 == 0:
                            nc.scalar.activation(
                                h_sb[:, m, :tw],
                                ph[:, :tw],
                                mybir.ActivationFunctionType.Relu,
                                bias=b18_sb[:, m : m + 1],
                            )
                        else:
                            nc.vector.tensor_scalar(
                                h_sb[:, m, :tw],
                                ph[:, :tw],
                                b18_sb[:, m : m + 1],
                                0.0,
                                add_op,
                                max_op,
                            )
                else:
                    h_sb = hpool.tile([P, MH, CHUNK], bf16, tag="h", name="h16")
                    for m in range(MH):
                        ph = pspool.tile([P, CHUNK], f32, tag="ps", name="ph")
                        for k in range(KD):
                            nc.tensor.matmul(
                                ph[:, :tw],
                                w1h_sb[:, k, ts(m, P)],
                                xt16_sb[:, k, off : off + tw],
                                start=(k == 0),
                                stop=(k == KD - 1),
                            )
                        nc.scalar.activation(
                            h_sb[:, m, :tw],
                            ph[:, :tw],
                            mybir.ActivationFunctionType.Relu,
                            bias=b1h_sb[:, m : m + 1],
                        )
                        if ci == 0 and m < 10:
                            # Fill DMA-ramp bubbles with dependency-free
                            # matmuls so the HAM clock gate stays at 8/8
                            # while chunk-0 weights stream in. Densest
                            # early, where arrival lags consumption most.
                            for _ in range(8 if m < 4 else 4):
                                nc.tensor.matmul(
                                    warm_ps, warm_sb, warm_sb,
                                    start=True, stop=True,
                                )
                mm2_phase(is8, off, tw, h_sb, last)
    nc.finalize()
    return nc


def _route(x, Wg):
    """Exact reference gating on host: top-2 of clean fp32 logits (jax
    tie-break: lower index first), softmax over the two picks."""
    logits = x @ Wg  # [B, E] fp32
    order = np.argsort(-logits, axis=1, kind="stable")[:, :TOP_K]  # [B, 2]
    top_vals = np.take_along_axis(logits, order, axis=1)
    ex = np.exp(top_vals - top_vals[:, :1])  # top_vals sorted desc -> max first
    gates = (ex / ex.sum(axis=1, keepdims=True)).astype(np.float32)  # [B, 2]
    return order, gates


def _pick_c16(counts, g2_prefix, total_g2):
    """Smallest C16 (multiple of 16) whose fp8 allocation n8[e] =
    max(0, counts[e] - C16) keeps the fp8 gate^2 mass under budget."""

    def frac(c16):
        used = 0.0
        for e in range(E):
            n8 = max(0, int(counts[e]) - c16)
            if n8:
                used += g2_prefix[e][min(n8, len(g2_prefix[e])) - 1]
        return used / total_g2

    # Prefer an exact multiple of 512 (bf16 chunks with no sub-512 tail).
    c512 = int(-(-counts.max() // 512) * 512) - 512
    while c512 > 0 and frac(c512) <= G2_BUDGET:
        c512 -= 512
    c512 += 512
    best = c512 if frac(c512) <= G2_BUDGET else None
    for c16 in range(1024, int(counts.max()) + 16, 16):
        if best is not None and c16 >= best:
            break
        if frac(c16) <= G2_BUDGET:
            # accept a non-512-multiple only if it saves >48 bf16 columns
            if best is None or best - c16 > 48:
                best = c16
            break
    if best is None:
        best = int(-(-int(counts.max()) // 16) * 16)
    return best


def _to_f8(a, scale):
    return np.clip(np.asarray(a, np.float32) * scale, -240.0, 240.0).astype(_F8)


def kernel(x, Wg, W1, b1, W2, b2):
    x = np.ascontiguousarray(np.asarray(x, dtype=np.float32))
    Wg = np.asarray(Wg, dtype=np.float32)
    W1 = np.asarray(W1, dtype=np.float32)
    b1 = np.asarray(b1, dtype=np.float32)
    W2 = np.asarray(W2, dtype=np.float32)
    b2 = np.asarray(b2, dtype=np.float32)

    order, gates = _route(x, Wg)

    expert_flat = order.reshape(-1)  # [2B]; pair p belongs to token p//2
    gate_flat = gates.reshape(-1)
    counts = np.bincount(expert_flat, minlength=E)

    # Per-expert pairs sorted by gate ascending (stable).
    pair_idx_of = []
    g2_prefix = []
    for e in range(E):
        pidx = np.where(expert_flat == e)[0]
        si = np.argsort(gate_flat[pidx], kind="stable")
        pidx = pidx[si]
        pair_idx_of.append(pidx)
        g2_prefix.append(np.cumsum(gate_flat[pidx].astype(np.float64) ** 2))
    total_g2 = float((gate_flat.astype(np.float64) ** 2).sum())

    C16 = _pick_c16(counts, g2_prefix, total_g2)
    n8 = np.maximum(0, counts - C16)
    C8 = int(-(-max(int(n8.max()), 16) // 16) * 16)  # pad to mult of 16

    # Per-pair placement for the combine step, and per-core inputs.
    core_of_pair = expert_flat  # core e == expert e
    col_of_pair = np.empty(2 * B, dtype=np.int64)
    is8_of_pair = np.zeros(2 * B, dtype=bool)
    xT = np.ascontiguousarray(x.T)  # [D, B]
    in_maps = []
    for e in range(E):
        pidx = pair_idx_of[e]
        p8, p16 = pidx[: n8[e]], pidx[n8[e] :]
        col_of_pair[p8] = np.arange(len(p8))
        is8_of_pair[p8] = True
        col_of_pair[p16] = np.arange(len(p16))

        xg16 = np.zeros((D, C16), dtype=_BF16)
        xg16[:, : len(p16)] = xT[:, p16 // 2].astype(_BF16)
        xg8 = np.zeros((D, C8), dtype=_F8)
        xg8[:, : len(p8)] = _to_f8(xT[:, p8 // 2], 1.0)

        in_maps.append({
            "xt16": xg16,
            "xt8": xg8,
            "w1h": W1[e].astype(_BF16),
            "w2h": W2[e].astype(_BF16),
            "w18": _to_f8(W1[e], S_W1),
            "w28": _to_f8(W2[e], S_W2),
            "b1h": np.ascontiguousarray(b1[e].reshape(MH, P).T),
            "b18": np.ascontiguousarray((S_H * b1[e]).reshape(MH, P).T),
        })

    nc = _build_program(C16, C8)

    from concourse.bass_utils import run_bass_kernel_spmd

    trace = os.environ.get("MOE_TRACE") == "1"
    kwargs = {}
    if trace:
        kwargs = dict(trace=True, trace_cores=list(range(E)))
    try:
        res = run_bass_kernel_spmd(nc, in_maps, core_ids=list(range(E)), **kwargs)
    except Exception:  # wedged accelerator: reset once and retry untraced
        try:
            import ctypes

            lib = ctypes.CDLL("/opt/axon/libaxon_pjrt.so")
            lib.axon_reset.restype = ctypes.c_int64
            lib.axon_reset()
        except OSError:
            pass
        res = run_bass_kernel_spmd(nc, in_maps, core_ids=list(range(E)))
    global LAST_RESULTS
    LAST_RESULTS = res

    Y16 = np.stack([r["yt16"] for r in res.results]).astype(np.float32)
    Y8 = np.stack([r["yt8"] for r in res.results]).astype(np.float32)
    Y8 *= 1.0 / (S_H * S_W2)

    # Combine: pair p contributes gate_p * (y[:, col_p] + b2[e_p]) to token
    # p//2. Pairs of token b sit at flat positions 2b, 2b+1.
    cols = np.empty((2 * B, D), dtype=np.float32)
    m8 = is8_of_pair
    cols[~m8] = Y16[core_of_pair[~m8], :, col_of_pair[~m8]]
    cols[m8] = Y8[core_of_pair[m8], :, col_of_pair[m8]]
    weighted = (cols + b2[expert_flat]) * gate_flat[:, None]
    out = weighted[0::2] + weighted[1::2]
    return np.ascontiguousarray(out, dtype=np.float32)
